# revision 33
# baseline (speedup 1.0000x reference)
"""GQA attention (B=2, T=2048, D=4096, H=32, G=8, d=128) on 8 TRN2 NeuronCores.

Sharding: one KV group per core (4 Q heads + 1 K/V head). Each core:
  - projects q/k/v for its group in transposed ("d-major") layout,
  - causal attention with transposed score tiles S.T = K.T-chunk @ Q-tile,
  - partial out-projection against its 512 columns of Wo.
Host sums the 8 partial outputs and adds bo.

Optimizations over the first working kernel (886 -> ~727 us):
  - ragged diagonal: the 4 diagonal k-tiles of each 512-token q-tile only
    compute/stream the causally-live columns (512/384/256/128 instead of
    4x512), with the 128x128 triangle zeroed by a DVE 0/1-mask multiply
    after exp (no more -1e9 mask matmuls on the PE),
  - softmax row-sums accumulated on the DVE (bf16 pair-sums + f32
    accumulator) with a single bf16 ones-matmul per (head, q-tile) for the
    partition-broadcast, replacing the per-k-tile ones-matmuls; reciprocal
    via the fast (~11-bit) DVE approximation,
  - out-projection interleaved into the attention instruction stream (per
    token-tile, as soon as its 4 heads are normalized) via a two-queue
    emission scheduler, so the PE never idles while ACT chews exp()s,
  - partial outputs staged and DMA'd in bf16 (halves the HBM write traffic;
    host accumulates in f32),
  - deep x-tile prefetch (4 bufs) and fine-grained first-tile DMA chunks so
    the k-projection never waits on HBM.

All matmuls in bf16 with fp32 PSUM accumulation.
"""

import math
import sys

import numpy as np

sys.path.insert(0, "/opt/trn_rl_repo")

import ml_dtypes

BF16 = ml_dtypes.bfloat16

B, T, D = 2, 2048, 4096
H, G, d = 32, 8, 128
GROUP = H // G  # 4 heads per group/core
NC_ = 8  # cores

TOK_TILE = 512  # q-token tile (free dim of score matmuls, psum bank)
DC = D // 128  # 32 contraction chunks

_program_cache = {}


def _build_program(T_=T):
    import concourse.mybir as mybir
    import concourse.tile as tile
    from concourse import bacc
    from concourse.bass import ds, ts
    from concourse.masks import make_upper_triangular

    f32 = mybir.dt.float32
    bf16 = mybir.dt.bfloat16
    AF = mybir.ActivationFunctionType

    NT_ = B * T_  # total tokens
    NTT_ = NT_ // TOK_TILE  # phase-1 token tiles
    KTB = T_ // 128  # k-tiles per batch
    NQI = T_ // TOK_TILE  # q-tiles per batch

    nc = bacc.Bacc()

    xt_d = nc.declare_dram_parameter("xt", [128, DC, NT_], bf16, isOutput=False)
    wq_d = nc.declare_dram_parameter("wq", [128, DC * GROUP, 128], bf16, isOutput=False)
    wk_d = nc.declare_dram_parameter("wk", [128, DC, 128], bf16, isOutput=False)
    wv_d = nc.declare_dram_parameter("wv", [128, DC, 128], bf16, isOutput=False)
    wo_d = nc.declare_dram_parameter("wo", [128, GROUP * DC, 128], bf16, isOutput=False)
    bq_d = nc.declare_dram_parameter("bq", [128, GROUP], f32, isOutput=False)
    bk_d = nc.declare_dram_parameter("bk", [128, 1], f32, isOutput=False)
    bv_d = nc.declare_dram_parameter("bv", [128, 1], f32, isOutput=False)
    out_d = nc.declare_dram_parameter("out", [128, DC, NT_], bf16, isOutput=True)

    with tile.TileContext(nc) as tc:
        with tc.tile_pool(name="persist", bufs=1) as persist:
            qT = persist.tile([128, GROUP, NT_], bf16)  # [dq_row, head, tok]
            kT = persist.tile([128, NT_], bf16)  # [d, tok]
            vtm = persist.tile([128, NT_ // 128, 128], bf16)  # [tok_in_tile, tile, dv]
            yT = persist.tile([128, GROUP, NT_], bf16)  # [dv, head, tok]
            bq_s = persist.tile([128, GROUP], f32)
            bk_s = persist.tile([128, 1], f32)
            bv_s = persist.tile([128, 1], f32)
            tri01 = persist.tile([128, 128], bf16)  # keep (1.0) where kp <= qo
            onesb = persist.tile([128, 128], bf16)

            warm = persist.tile([128, 128], bf16)

            nc.vector.memset(warm[:], 0.0)
            make_upper_triangular(nc, tri01[:], val=1.0, diag=True)
            nc.vector.memset(onesb[:], 1.0)

            # ---------------- Phase 1: q/k/v projections ----------------
            with (
                tc.tile_pool(name="wproj", bufs=1) as wpool,
                tc.tile_pool(name="xtp", bufs=4) as xpool,
                tc.tile_pool(name="vtstage", bufs=3) as vtp,
                tc.psum_pool(name="ps1", bufs=5) as ps1,
            ):
                wk_s = wpool.tile([128, DC, 128], bf16)
                wv_s = wpool.tile([128, DC, 128], bf16)
                wq_s = wpool.tile([128, GROUP * DC, 128], bf16)
                HDC = DC // 2  # 16: stream x in two half-D chunks to fit SBUF

                # interleave k-weight and first-x chunks so the PE's k-proj
                # can start (and keep running) as early as possible.
                xa0 = xpool.tile([128, HDC, TOK_TILE], bf16, tag="xt")
                xb0 = xpool.tile([128, HDC, TOK_TILE], bf16, tag="xt")
                QC = DC // 4
                HQC = HDC // 4
                for ci in range(4):
                    nc.sync.dma_start(
                        out=wk_s[:, ts(ci, QC), :], in_=wk_d[:, ts(ci, QC), :]
                    )
                    nc.sync.dma_start(
                        out=xa0[:, ts(ci, HQC), :],
                        in_=xt_d[:, ds(ci * HQC, HQC), ts(0, TOK_TILE)],
                    )
                for ci in range(4):
                    nc.sync.dma_start(
                        out=xb0[:, ts(ci, HQC), :],
                        in_=xt_d[:, ds(HDC + ci * HQC, HQC), ts(0, TOK_TILE)],
                    )
                nc.sync.dma_start(out=bk_s[:], in_=bk_d[:])
                nc.sync.dma_start(out=wv_s[:], in_=wv_d[:])
                nc.sync.dma_start(out=bv_s[:], in_=bv_d[:])
                nc.sync.dma_start(
                    out=wq_s[:, ts(0, DC), :], in_=wq_d[:, ts(0, DC), :]
                )
                nc.sync.dma_start(out=bq_s[:], in_=bq_d[:])
                for dq in range(1, GROUP):
                    nc.sync.dma_start(
                        out=wq_s[:, ts(dq, DC), :], in_=wq_d[:, ts(dq, DC), :]
                    )

                # keep the PE's HAM activity window busy while the first
                # x/weight chunks stream in, so real matmuls start at 2.4 GHz
                wps = ps1.tile([128, TOK_TILE], f32, name="wps", bufs=1)
                for _ in range(60):
                    nc.tensor.matmul(
                        wps[:, 0:128], lhsT=warm[:], rhs=warm[:],
                        start=True, stop=True,
                    )

                for tt in range(NTT_):
                    if tt == 0:
                        xa, xb = xa0, xb0
                    else:
                        xa = xpool.tile([128, HDC, TOK_TILE], bf16, tag="xt")
                        xb = xpool.tile([128, HDC, TOK_TILE], bf16, tag="xt")
                        nc.sync.dma_start(
                            out=xa[:], in_=xt_d[:, 0:HDC, ts(tt, TOK_TILE)]
                        )
                        nc.sync.dma_start(
                            out=xb[:], in_=xt_d[:, HDC:DC, ts(tt, TOK_TILE)]
                        )

                    def xsrc(Dc):
                        return (xa if Dc < HDC else xb)[:, Dc % HDC, :]

                    ps = ps1.tile([128, TOK_TILE], f32)
                    for Dc in range(DC):
                        nc.tensor.matmul(
                            ps[:],
                            lhsT=wk_s[:, Dc, :],
                            rhs=xsrc(Dc),
                            start=(Dc == 0),
                            stop=(Dc == DC - 1),
                        )
                    nc.scalar.activation(
                        out=kT[:, ts(tt, TOK_TILE)],
                        in_=ps[:],
                        func=AF.Identity,
                        bias=bk_s[:, 0:1],
                    )
                    ps = ps1.tile([128, TOK_TILE], f32)
                    for Dc in range(DC):
                        nc.tensor.matmul(
                            ps[:],
                            lhsT=wv_s[:, Dc, :],
                            rhs=xsrc(Dc),
                            start=(Dc == 0),
                            stop=(Dc == DC - 1),
                        )
                    vt_t = vtp.tile([128, TOK_TILE], bf16)
                    nc.scalar.activation(
                        out=vt_t[:],
                        in_=ps[:],
                        func=AF.Identity,
                        bias=bv_s[:, 0:1],
                    )

                    def qgroup(dq):
                        ps = ps1.tile([128, TOK_TILE], f32, name="ps")
                        for Dc in range(DC):
                            nc.tensor.matmul(
                                ps[:],
                                lhsT=wq_s[:, dq * DC + Dc, :],
                                rhs=xsrc(Dc),
                                start=(Dc == 0),
                                stop=(Dc == DC - 1),
                            )
                        nc.scalar.activation(
                            out=qT[:, dq, ts(tt, TOK_TILE)],
                            in_=ps[:],
                            func=AF.Identity,
                            bias=bq_s[:, dq : dq + 1],
                        )

                    qgroup(0)
                    # transpose this tile's v to token-major on the (idle)
                    # DMA engines via the XBAR: out[p, m, d] = v[d, m*128+p]
                    nc.sync.dma_start(
                        out=vtm[:, ds(tt * 4, 4), :], in_=vt_t[:], transpose=True
                    )
                    for dq in range(1, GROUP):
                        qgroup(dq)

            # ------------- Phase 2+3: attention + out-projection -------------
            with tc.tile_pool(name="wout", bufs=1) as wop:
                wo_s = wop.tile([128, GROUP * DC, 128], bf16)
                for c in range(GROUP):
                    nc.sync.dma_start(
                        out=wo_s[:, ts(c, DC), :], in_=wo_d[:, ts(c, DC), :]
                    )

                with (
                    tc.psum_pool(name="ps2", bufs=1) as ps2,
                    tc.tile_pool(name="ptile", bufs=1) as ppool,
                    tc.tile_pool(name="lacc", bufs=3) as laccp,
                    tc.tile_pool(name="scr", bufs=2) as scrp,
                    tc.tile_pool(name="invl", bufs=2) as invp,
                    tc.tile_pool(name="stg", bufs=4) as stg,
                ):
                    W2 = 2 * TOK_TILE

                    # ---- emission scheduler: score stages paced against
                    # ---- filler PE work. Two queues: "attn" units (attnV /
                    # ---- finish — dependent on just-produced exps) alternate
                    # ---- with "proj" units (out-proj — dependency-free), so
                    # ---- a not-yet-ready attn unit never heads the PE FIFO
                    # ---- with nothing in front of it.
                    fq_attn = []
                    fq_proj = []
                    state = {"ai": 0, "pi": 0}

                    def emit_filler(n):
                        k = 0
                        while k < n:
                            a = state["ai"] < len(fq_attn)
                            # a proj unit reads yT written by a finish unit;
                            # emission order IS the dep-tracking order, so it
                            # may only go out once that finish was emitted.
                            p = (
                                state["pi"] < len(fq_proj)
                                and fq_proj[state["pi"]][0] <= state["ai"]
                            )
                            if not a and not p:
                                break
                            if a:
                                fq_attn[state["ai"]]()
                                state["ai"] += 1
                                k += 1
                                p = (
                                    state["pi"] < len(fq_proj)
                                    and fq_proj[state["pi"]][0] <= state["ai"]
                                )
                            if p and k < n:
                                fq_proj[state["pi"]][1]()
                                state["pi"] += 1
                                k += 1

                    def drain_filler():
                        while (
                            state["ai"] < len(fq_attn) or state["pi"] < len(fq_proj)
                        ):
                            emit_filler(2)

                    def get_yps(it):
                        if it["yps"] is None:
                            it["yps"] = ps2.tile(
                                [128, TOK_TILE], f32, tag="yps", bufs=2, name="yps"
                            )
                        return it["yps"]

                    def get_lacc(it):
                        if it["lacc"] is None:
                            it["lacc"] = laccp.tile([128, TOK_TILE], f32, name="lacc")
                        return it["lacc"]

                    def mk_pair(it, jp):
                        def f():
                            b, h, qi = it["b"], it["h"], it["qi"]
                            q0 = b * T_ + qi * TOK_TILE
                            st = ps2.tile([128, W2], f32, tag="st", bufs=2, name="st")
                            pt = ppool.tile([128, W2], bf16, tag="pt", bufs=16, name="pt")
                            for jj in range(2):
                                j = 2 * jp + jj
                                nc.tensor.matmul(
                                    st[:, ds(jj * TOK_TILE, TOK_TILE)],
                                    lhsT=kT[:, ds(b * T_ + j * 128, 128)],
                                    rhs=qT[:, h, ds(q0, TOK_TILE)],
                                    start=True,
                                    stop=True,
                                )
                            nc.scalar.activation(out=pt[:], in_=st[:], func=AF.Exp)
                            it["p"].append(pt)

                        return f

                    def mk_diagA(it):
                        def f():
                            b, h, qi = it["b"], it["h"], it["qi"]
                            q0 = b * T_ + qi * TOK_TILE
                            kb = 4 * qi
                            st = ps2.tile([128, W2], f32, tag="st", bufs=2, name="st")
                            pd = ppool.tile([128, W2], bf16, tag="pt", bufs=16, name="pd")
                            nc.tensor.matmul(
                                st[:, ds(0, 512)],
                                lhsT=kT[:, ds(b * T_ + kb * 128, 128)],
                                rhs=qT[:, h, ds(q0, 512)],
                                start=True,
                                stop=True,
                            )
                            nc.tensor.matmul(
                                st[:, ds(512, 384)],
                                lhsT=kT[:, ds(b * T_ + (kb + 1) * 128, 128)],
                                rhs=qT[:, h, ds(q0 + 128, 384)],
                                start=True,
                                stop=True,
                            )
                            nc.scalar.activation(
                                out=pd[:, ds(0, 896)], in_=st[:, ds(0, 896)], func=AF.Exp
                            )
                            it["pdA"] = pd

                        return f

                    def mk_diagB(it):
                        def f():
                            b, h, qi = it["b"], it["h"], it["qi"]
                            q0 = b * T_ + qi * TOK_TILE
                            kb = 4 * qi
                            st = ps2.tile([128, W2], f32, tag="st", bufs=2, name="st")
                            pd = ppool.tile([128, W2], bf16, tag="pt", bufs=16, name="pd")
                            nc.tensor.matmul(
                                st[:, ds(0, 256)],
                                lhsT=kT[:, ds(b * T_ + (kb + 2) * 128, 128)],
                                rhs=qT[:, h, ds(q0 + 256, 256)],
                                start=True,
                                stop=True,
                            )
                            nc.tensor.matmul(
                                st[:, ds(256, 128)],
                                lhsT=kT[:, ds(b * T_ + (kb + 3) * 128, 128)],
                                rhs=qT[:, h, ds(q0 + 384, 128)],
                                start=True,
                                stop=True,
                            )
                            nc.scalar.activation(
                                out=pd[:, ds(0, 384)], in_=st[:, ds(0, 384)], func=AF.Exp
                            )
                            it["pdB"] = pd

                        return f

                    def mk_av_pair(it, jp):
                        def f():
                            b, qi = it["b"], it["qi"]
                            pt = it["p"][jp]
                            yps = get_yps(it)
                            for jj in range(2):
                                j = 2 * jp + jj
                                nc.tensor.matmul(
                                    yps[:],
                                    lhsT=vtm[:, b * KTB + j, :],
                                    rhs=pt[:, ds(jj * TOK_TILE, TOK_TILE)],
                                    start=(j == 0),
                                    stop=False,
                                )
                            lacc = get_lacc(it)
                            if jp == 0:
                                nc.vector.tensor_add(
                                    out=lacc[:],
                                    in0=pt[:, ds(0, 512)],
                                    in1=pt[:, ds(512, 512)],
                                )
                            else:
                                sc = scrp.tile([128, TOK_TILE], bf16, name="sc")
                                nc.vector.tensor_add(
                                    out=sc[:],
                                    in0=pt[:, ds(0, 512)],
                                    in1=pt[:, ds(512, 512)],
                                )
                                nc.vector.tensor_add(
                                    out=lacc[:], in0=lacc[:], in1=sc[:]
                                )

                        return f

                    def mk_av_diag(it):
                        def f():
                            b, qi = it["b"], it["qi"]
                            kb = 4 * qi
                            pA, pB = it["pdA"], it["pdB"]
                            # zero the causally-dead triangle (kp > qo)
                            nc.vector.tensor_mul(
                                out=pA[:, ds(0, 128)], in0=pA[:, ds(0, 128)], in1=tri01[:]
                            )
                            nc.vector.tensor_mul(
                                out=pA[:, ds(512, 128)],
                                in0=pA[:, ds(512, 128)],
                                in1=tri01[:],
                            )
                            nc.vector.tensor_mul(
                                out=pB[:, ds(0, 128)], in0=pB[:, ds(0, 128)], in1=tri01[:]
                            )
                            nc.vector.tensor_mul(
                                out=pB[:, ds(256, 128)],
                                in0=pB[:, ds(256, 128)],
                                in1=tri01[:],
                            )
                            yps = get_yps(it)
                            nc.tensor.matmul(
                                yps[:],
                                lhsT=vtm[:, b * KTB + kb, :],
                                rhs=pA[:, ds(0, 512)],
                                start=(qi == 0),
                                stop=False,
                            )
                            nc.tensor.matmul(
                                yps[:, ds(128, 384)],
                                lhsT=vtm[:, b * KTB + kb + 1, :],
                                rhs=pA[:, ds(512, 384)],
                                start=False,
                                stop=False,
                            )
                            nc.tensor.matmul(
                                yps[:, ds(256, 256)],
                                lhsT=vtm[:, b * KTB + kb + 2, :],
                                rhs=pB[:, ds(0, 256)],
                                start=False,
                                stop=False,
                            )
                            nc.tensor.matmul(
                                yps[:, ds(384, 128)],
                                lhsT=vtm[:, b * KTB + kb + 3, :],
                                rhs=pB[:, ds(256, 128)],
                                start=False,
                                stop=True,
                            )
                            lacc = get_lacc(it)
                            if it["qi"] == 0:
                                nc.vector.tensor_copy(
                                    out=lacc[:], in_=pA[:, ds(0, 512)]
                                )
                            else:
                                nc.vector.tensor_add(
                                    out=lacc[:], in0=lacc[:], in1=pA[:, ds(0, 512)]
                                )
                            nc.vector.tensor_add(
                                out=lacc[:, ds(128, 384)],
                                in0=lacc[:, ds(128, 384)],
                                in1=pA[:, ds(512, 384)],
                            )
                            nc.vector.tensor_add(
                                out=lacc[:, ds(256, 256)],
                                in0=lacc[:, ds(256, 256)],
                                in1=pB[:, ds(0, 256)],
                            )
                            nc.vector.tensor_add(
                                out=lacc[:, ds(384, 128)],
                                in0=lacc[:, ds(384, 128)],
                                in1=pB[:, ds(256, 128)],
                            )

                        return f

                    def mk_finish(it):
                        def f():
                            b, h, qi = it["b"], it["h"], it["qi"]
                            lb = scrp.tile(
                                [128, TOK_TILE], bf16, tag="lb16", name="lb"
                            )
                            nc.scalar.copy(out=lb[:], in_=it["lacc"][:])
                            lps = ps2.tile([128, TOK_TILE], f32, tag="ops", bufs=2, name="lps")
                            nc.tensor.matmul(
                                lps[:],
                                lhsT=onesb[:],
                                rhs=lb[:],
                                start=True,
                                stop=True,
                            )
                            inv = invp.tile([128, TOK_TILE], f32, tag="inv", name="inv")
                            # ~11-bit reciprocal is ample: l already carries
                            # bf16 quantization noise an order larger.
                            nc.vector.reciprocal_approx_fast(out=inv[:], in_=lps[:])
                            nc.vector.tensor_mul(
                                out=yT[:, h, ds(b * T_ + qi * TOK_TILE, TOK_TILE)],
                                in0=it["yps"][:],
                                in1=inv[:],
                            )

                        return f

                    def mk_outproj(tt, Do):
                        def f():
                            ps = ps2.tile([128, TOK_TILE], f32, tag="ops", bufs=2, name="ops")
                            for c in range(GROUP):
                                nc.tensor.matmul(
                                    ps[:],
                                    lhsT=wo_s[:, c * DC + Do, :],
                                    rhs=yT[:, c, ts(tt, TOK_TILE)],
                                    start=(c == 0),
                                    stop=(c == GROUP - 1),
                                )
                            so = stg.tile([128, TOK_TILE], bf16, name="so")
                            if Do % 2 == 0:
                                nc.vector.tensor_copy(out=so[:], in_=ps[:])
                            else:
                                nc.scalar.copy(out=so[:], in_=ps[:])
                            nc.sync.dma_start(
                                out=out_d[:, Do, ts(tt, TOK_TILE)], in_=so[:]
                            )

                        return f

                    for b in range(B):
                        # b1 descends so the kernel ends on the lightest
                        # group (shortest finish chains before the drain)
                        qorder = (
                            range(NQI) if b == 0 else range(NQI - 1, -1, -1)
                        )
                        for qi in qorder:
                            for h in range(GROUP):
                                it = {
                                    "b": b,
                                    "h": h,
                                    "qi": qi,
                                    "p": [],
                                    "pdA": None,
                                    "pdB": None,
                                    "yps": None,
                                    "lacc": None,
                                }
                                stages = [mk_pair(it, jp) for jp in range(2 * qi)]
                                stages.append(mk_diagA(it))
                                stages.append(mk_diagB(it))
                                for s in stages:
                                    s()
                                    emit_filler(2)
                                for jp in range(2 * qi):
                                    fq_attn.append(mk_av_pair(it, jp))
                                fq_attn.append(mk_av_diag(it))
                                fq_attn.append(mk_finish(it))
                            # this q-tile's out-projection becomes available
                            # once its 4 finish units are emitted; queue it.
                            tt = b * NQI + qi
                            need = len(fq_attn)
                            for Do in range(DC):
                                fq_proj.append((need, mk_outproj(tt, Do)))
                    drain_filler()

    if not nc.is_finalized():
        nc.finalize()
    return nc


def _prep_inputs(hidden_states, Wq, bq, Wk, bk, Wv, bv, Wo, bo, T_=T):
    NT_ = B * T_
    scale = 1.0 / math.sqrt(d)

    x_flat = np.asarray(hidden_states, dtype=np.float32).reshape(NT_, D)
    # xt[p, Dc, t] = x[t, Dc*128+p]
    xt = np.ascontiguousarray(
        x_flat.reshape(NT_, DC, 128).transpose(2, 1, 0)
    ).astype(BF16)

    in_maps = []
    for g in range(NC_):
        Wq_g = np.asarray(Wq[g * 512 : (g + 1) * 512, :], dtype=np.float32) * scale
        bq_g = np.asarray(bq[g * 512 : (g + 1) * 512], dtype=np.float32) * scale
        Wk_g = np.asarray(Wk[g * 128 : (g + 1) * 128, :], dtype=np.float32)
        bk_g = np.asarray(bk[g * 128 : (g + 1) * 128], dtype=np.float32)
        Wv_g = np.asarray(Wv[g * 128 : (g + 1) * 128, :], dtype=np.float32)
        bv_g = np.asarray(bv[g * 128 : (g + 1) * 128], dtype=np.float32)
        Wo_g = np.asarray(Wo[:, g * 512 : (g + 1) * 512], dtype=np.float32)

        # wq[p, dq*DC+Dc, m] = Wq_g[dq*128+m, Dc*128+p]
        wq_host = np.ascontiguousarray(
            Wq_g.reshape(GROUP, 128, DC, 128).transpose(3, 0, 2, 1).reshape(
                128, GROUP * DC, 128
            )
        ).astype(BF16)
        # wk[p, Dc, m] = Wk_g[m, Dc*128+p]
        wk_host = np.ascontiguousarray(
            Wk_g.reshape(128, DC, 128).transpose(2, 1, 0)
        ).astype(BF16)
        wv_host = np.ascontiguousarray(
            Wv_g.reshape(128, DC, 128).transpose(2, 1, 0)
        ).astype(BF16)
        # wo[p, c*DC+Do, m] = Wo_g[Do*128+m, c*128+p]
        wo_host = np.ascontiguousarray(
            Wo_g.reshape(DC, 128, GROUP, 128).transpose(3, 2, 0, 1).reshape(
                128, GROUP * DC, 128
            )
        ).astype(BF16)

        in_maps.append(
            {
                "xt": xt,
                "wq": wq_host,
                "wk": wk_host,
                "wv": wv_host,
                "wo": wo_host,
                "bq": np.ascontiguousarray(bq_g.reshape(GROUP, 128).T),
                "bk": bk_g.reshape(128, 1).copy(),
                "bv": bv_g.reshape(128, 1).copy(),
            }
        )
    return in_maps


def kernel(
    hidden_states, Wq, bq, Wk, bk, Wv, bv, Wo, bo, _trace=False, _result_box=None
):
    from concourse.bass_utils import run_bass_kernel_spmd

    if "nc" not in _program_cache:
        _program_cache["nc"] = _build_program()
    nc = _program_cache["nc"]

    in_maps = _prep_inputs(hidden_states, Wq, bq, Wk, bk, Wv, bv, Wo, bo)
    res = run_bass_kernel_spmd(
        nc, in_maps, core_ids=list(range(NC_)), trace=_trace
    )
    if _result_box is not None:
        _result_box.append(res)

    NT_ = B * T
    acc = np.zeros((128, DC, NT_), dtype=np.float32)
    for r in res.results:
        acc += r["out"].astype(np.float32)
    # outT[Do*128+p, t] = acc[p, Do, t];  out[t, :] = outT[:, t] + bo
    outT = acc.transpose(1, 0, 2).reshape(D, NT_)
    out = outT.T + np.asarray(bo, dtype=np.float32)[None, :]
    return np.ascontiguousarray(out.reshape(B, T, D), dtype=np.float32)


# revision 34
# speedup vs baseline: 1.0633x; 1.0633x over previous
"""GQA attention (B=2, T=2048, D=4096, H=32, G=8, d=128) on 8 TRN2 NeuronCores.

Sharding: one KV group per core (4 Q heads + 1 K/V head). Each core:
  - projects q/k/v for its group in transposed ("d-major") layout,
  - causal attention with transposed score tiles S.T = K.T-chunk @ Q-tile,
  - partial out-projection against its 512 columns of Wo.
Host sums the 8 partial outputs and adds bo.

Optimizations over the first working kernel (886 -> ~727 us):
  - ragged diagonal: the 4 diagonal k-tiles of each 512-token q-tile only
    compute/stream the causally-live columns (512/384/256/128 instead of
    4x512), with the 128x128 triangle zeroed by a DVE 0/1-mask multiply
    after exp (no more -1e9 mask matmuls on the PE),
  - softmax row-sums accumulated on the DVE (bf16 pair-sums + f32
    accumulator) with a single bf16 ones-matmul per (head, q-tile) for the
    partition-broadcast, replacing the per-k-tile ones-matmuls; reciprocal
    via the fast (~11-bit) DVE approximation,
  - out-projection interleaved into the attention instruction stream (per
    token-tile, as soon as its 4 heads are normalized) via a two-queue
    emission scheduler, so the PE never idles while ACT chews exp()s,
  - partial outputs staged and DMA'd in bf16 (halves the HBM write traffic;
    host accumulates in f32),
  - deep x-tile prefetch (4 bufs) and fine-grained first-tile DMA chunks so
    the k-projection never waits on HBM.

All matmuls in bf16 with fp32 PSUM accumulation.
"""

import math
import sys

import numpy as np

sys.path.insert(0, "/opt/trn_rl_repo")

import ml_dtypes

BF16 = ml_dtypes.bfloat16

B, T, D = 2, 2048, 4096
H, G, d = 32, 8, 128
GROUP = H // G  # 4 heads per group/core
NC_ = 8  # cores

TOK_TILE = 512  # q-token tile (free dim of score matmuls, psum bank)
DC = D // 128  # 32 contraction chunks

_program_cache = {}


def _build_program(T_=T):
    import concourse.mybir as mybir
    import concourse.tile as tile
    from concourse import bacc
    from concourse.bass import ds, ts
    from concourse.masks import make_upper_triangular

    f32 = mybir.dt.float32
    bf16 = mybir.dt.bfloat16
    AF = mybir.ActivationFunctionType

    NT_ = B * T_  # total tokens
    NTT_ = NT_ // TOK_TILE  # phase-1 token tiles
    KTB = T_ // 128  # k-tiles per batch
    NQI = T_ // TOK_TILE  # q-tiles per batch

    nc = bacc.Bacc()

    xt_d = nc.declare_dram_parameter("xt", [128, DC, NT_], bf16, isOutput=False)
    wq_d = nc.declare_dram_parameter("wq", [128, DC * GROUP, 128], bf16, isOutput=False)
    wk_d = nc.declare_dram_parameter("wk", [128, DC, 128], bf16, isOutput=False)
    wv_d = nc.declare_dram_parameter("wv", [128, DC, 128], bf16, isOutput=False)
    wo_d = nc.declare_dram_parameter("wo", [128, GROUP * DC, 128], bf16, isOutput=False)
    bq_d = nc.declare_dram_parameter("bq", [128, GROUP], f32, isOutput=False)
    bk_d = nc.declare_dram_parameter("bk", [128, 1], f32, isOutput=False)
    bv_d = nc.declare_dram_parameter("bv", [128, 1], f32, isOutput=False)
    out_d = nc.declare_dram_parameter("out", [128, DC, NT_], bf16, isOutput=True)

    with tile.TileContext(nc) as tc:
        with tc.tile_pool(name="persist", bufs=1) as persist:
            qT = persist.tile([128, GROUP, NT_], bf16)  # [dq_row, head, tok]
            kT = persist.tile([128, NT_], bf16)  # [d, tok]
            vtm = persist.tile([128, NT_ // 128, 128], bf16)  # [tok_in_tile, tile, dv]
            yT = persist.tile([128, GROUP, NT_], bf16)  # [dv, head, tok]
            bq_s = persist.tile([128, GROUP], f32)
            bk_s = persist.tile([128, 1], f32)
            bv_s = persist.tile([128, 1], f32)
            tri01 = persist.tile([128, 128], bf16)  # keep (1.0) where kp <= qo
            onesb = persist.tile([128, 128], bf16)

            warm = persist.tile([128, 128], bf16)

            nc.vector.memset(warm[:], 0.0)
            make_upper_triangular(nc, tri01[:], val=1.0, diag=True)
            nc.vector.memset(onesb[:], 1.0)

            # ---------------- Phase 1: q/k/v projections ----------------
            with (
                tc.tile_pool(name="wproj", bufs=1) as wpool,
                tc.tile_pool(name="xtp", bufs=4) as xpool,
                tc.tile_pool(name="vtstage", bufs=2) as vtp,
                tc.psum_pool(name="ps1", bufs=4) as ps1,
            ):
                wk_s = wpool.tile([128, DC, 128], bf16)
                wv_s = wpool.tile([128, DC, 128], bf16)
                wq_s = wpool.tile([128, GROUP * DC, 128], bf16)
                HDC = DC // 2  # 16: stream x in two half-D chunks to fit SBUF

                # interleave k-weight and first-x chunks so the PE's k-proj
                # can start (and keep running) as early as possible.
                xa0 = xpool.tile([128, HDC, TOK_TILE], bf16, tag="xt")
                xb0 = xpool.tile([128, HDC, TOK_TILE], bf16, tag="xt")
                QC = DC // 4
                HQC = HDC // 4
                for ci in range(4):
                    nc.sync.dma_start(
                        out=wk_s[:, ts(ci, QC), :], in_=wk_d[:, ts(ci, QC), :]
                    )
                    nc.sync.dma_start(
                        out=xa0[:, ts(ci, HQC), :],
                        in_=xt_d[:, ds(ci * HQC, HQC), ts(0, TOK_TILE)],
                    )
                for ci in range(4):
                    nc.sync.dma_start(
                        out=xb0[:, ts(ci, HQC), :],
                        in_=xt_d[:, ds(HDC + ci * HQC, HQC), ts(0, TOK_TILE)],
                    )
                nc.sync.dma_start(out=bk_s[:], in_=bk_d[:])
                nc.sync.dma_start(out=wv_s[:], in_=wv_d[:])
                nc.sync.dma_start(out=bv_s[:], in_=bv_d[:])
                nc.sync.dma_start(
                    out=wq_s[:, ts(0, DC), :], in_=wq_d[:, ts(0, DC), :]
                )
                nc.sync.dma_start(out=bq_s[:], in_=bq_d[:])
                for dq in range(1, GROUP):
                    nc.sync.dma_start(
                        out=wq_s[:, ts(dq, DC), :], in_=wq_d[:, ts(dq, DC), :]
                    )

                # keep the PE's HAM activity window busy while the first
                # x/weight chunks stream in, so real matmuls start at 2.4 GHz
                wps = ps1.tile([128, TOK_TILE], f32, name="wps")
                for _ in range(60):
                    nc.tensor.matmul(
                        wps[:, 0:128], lhsT=warm[:], rhs=warm[:],
                        start=True, stop=True,
                    )

                for tt in range(NTT_):
                    if tt == 0:
                        xa, xb = xa0, xb0
                    else:
                        xa = xpool.tile([128, HDC, TOK_TILE], bf16, tag="xt")
                        xb = xpool.tile([128, HDC, TOK_TILE], bf16, tag="xt")
                        nc.sync.dma_start(
                            out=xa[:], in_=xt_d[:, 0:HDC, ts(tt, TOK_TILE)]
                        )
                        nc.sync.dma_start(
                            out=xb[:], in_=xt_d[:, HDC:DC, ts(tt, TOK_TILE)]
                        )

                    def xsrc(Dc):
                        return (xa if Dc < HDC else xb)[:, Dc % HDC, :]

                    ps = ps1.tile([128, TOK_TILE], f32)
                    for Dc in range(DC):
                        nc.tensor.matmul(
                            ps[:],
                            lhsT=wk_s[:, Dc, :],
                            rhs=xsrc(Dc),
                            start=(Dc == 0),
                            stop=(Dc == DC - 1),
                        )
                    nc.scalar.activation(
                        out=kT[:, ts(tt, TOK_TILE)],
                        in_=ps[:],
                        func=AF.Identity,
                        bias=bk_s[:, 0:1],
                    )
                    ps = ps1.tile([128, TOK_TILE], f32)
                    for Dc in range(DC):
                        nc.tensor.matmul(
                            ps[:],
                            lhsT=wv_s[:, Dc, :],
                            rhs=xsrc(Dc),
                            start=(Dc == 0),
                            stop=(Dc == DC - 1),
                        )
                    vt_t = vtp.tile([128, TOK_TILE], bf16)
                    nc.scalar.activation(
                        out=vt_t[:],
                        in_=ps[:],
                        func=AF.Identity,
                        bias=bv_s[:, 0:1],
                    )

                    def qgroup(dq):
                        ps = ps1.tile([128, TOK_TILE], f32, name="ps")
                        for Dc in range(DC):
                            nc.tensor.matmul(
                                ps[:],
                                lhsT=wq_s[:, dq * DC + Dc, :],
                                rhs=xsrc(Dc),
                                start=(Dc == 0),
                                stop=(Dc == DC - 1),
                            )
                        nc.scalar.activation(
                            out=qT[:, dq, ts(tt, TOK_TILE)],
                            in_=ps[:],
                            func=AF.Identity,
                            bias=bq_s[:, dq : dq + 1],
                        )

                    qgroup(0)
                    # transpose this tile's v to token-major on the (idle)
                    # DMA engines via the XBAR: out[p, m, d] = v[d, m*128+p]
                    nc.sync.dma_start(
                        out=vtm[:, ds(tt * 4, 4), :], in_=vt_t[:], transpose=True
                    )
                    for dq in range(1, GROUP):
                        qgroup(dq)

            # ------------- Phase 2+3: attention + out-projection -------------
            with tc.tile_pool(name="wout", bufs=1) as wop:
                wo_s = wop.tile([128, GROUP * DC, 128], bf16)
                for c in range(GROUP):
                    nc.sync.dma_start(
                        out=wo_s[:, ts(c, DC), :], in_=wo_d[:, ts(c, DC), :]
                    )

                with (
                    tc.psum_pool(name="ps2", bufs=1) as ps2,
                    tc.tile_pool(name="ptile", bufs=1) as ppool,
                    tc.tile_pool(name="lacc", bufs=3) as laccp,
                    tc.tile_pool(name="scr", bufs=2) as scrp,
                    tc.tile_pool(name="invl", bufs=2) as invp,
                    tc.tile_pool(name="stg", bufs=4) as stg,
                ):
                    W2 = 2 * TOK_TILE

                    # ---- emission scheduler: score stages paced against
                    # ---- filler PE work. Two queues: "attn" units (attnV /
                    # ---- finish — dependent on just-produced exps) alternate
                    # ---- with "proj" units (out-proj — dependency-free), so
                    # ---- a not-yet-ready attn unit never heads the PE FIFO
                    # ---- with nothing in front of it.
                    fq_attn = []
                    fq_proj = []
                    state = {"ai": 0, "pi": 0}

                    def emit_filler(n):
                        k = 0
                        while k < n:
                            a = state["ai"] < len(fq_attn)
                            # a proj unit reads yT written by a finish unit;
                            # emission order IS the dep-tracking order, so it
                            # may only go out once that finish was emitted.
                            p = (
                                state["pi"] < len(fq_proj)
                                and fq_proj[state["pi"]][0] <= state["ai"]
                            )
                            if not a and not p:
                                break
                            if a:
                                fq_attn[state["ai"]]()
                                state["ai"] += 1
                                k += 1
                                p = (
                                    state["pi"] < len(fq_proj)
                                    and fq_proj[state["pi"]][0] <= state["ai"]
                                )
                            if p and k < n:
                                fq_proj[state["pi"]][1]()
                                state["pi"] += 1
                                k += 1

                    def drain_filler():
                        while (
                            state["ai"] < len(fq_attn) or state["pi"] < len(fq_proj)
                        ):
                            emit_filler(2)

                    def get_yps(it):
                        if it["yps"] is None:
                            it["yps"] = ps2.tile(
                                [128, TOK_TILE], f32, tag="yps", bufs=2, name="yps"
                            )
                        return it["yps"]

                    def get_lacc(it):
                        if it["lacc"] is None:
                            it["lacc"] = laccp.tile([128, TOK_TILE], f32, name="lacc")
                        return it["lacc"]

                    def mk_pair(it, jp):
                        def f():
                            b, h, qi = it["b"], it["h"], it["qi"]
                            q0 = b * T_ + qi * TOK_TILE
                            st = ps2.tile([128, W2], f32, tag="st", bufs=2, name="st")
                            pt = ppool.tile([128, W2], bf16, tag="pt", bufs=16, name="pt")
                            for jj in range(2):
                                j = 2 * jp + jj
                                nc.tensor.matmul(
                                    st[:, ds(jj * TOK_TILE, TOK_TILE)],
                                    lhsT=kT[:, ds(b * T_ + j * 128, 128)],
                                    rhs=qT[:, h, ds(q0, TOK_TILE)],
                                    start=True,
                                    stop=True,
                                )
                            nc.scalar.activation(out=pt[:], in_=st[:], func=AF.Exp)
                            it["p"].append(pt)

                        return f

                    def mk_diagA(it):
                        def f():
                            b, h, qi = it["b"], it["h"], it["qi"]
                            q0 = b * T_ + qi * TOK_TILE
                            kb = 4 * qi
                            st = ps2.tile([128, W2], f32, tag="st", bufs=2, name="st")
                            pd = ppool.tile([128, W2], bf16, tag="pt", bufs=16, name="pd")
                            nc.tensor.matmul(
                                st[:, ds(0, 512)],
                                lhsT=kT[:, ds(b * T_ + kb * 128, 128)],
                                rhs=qT[:, h, ds(q0, 512)],
                                start=True,
                                stop=True,
                            )
                            nc.tensor.matmul(
                                st[:, ds(512, 384)],
                                lhsT=kT[:, ds(b * T_ + (kb + 1) * 128, 128)],
                                rhs=qT[:, h, ds(q0 + 128, 384)],
                                start=True,
                                stop=True,
                            )
                            nc.scalar.activation(
                                out=pd[:, ds(0, 896)], in_=st[:, ds(0, 896)], func=AF.Exp
                            )
                            it["pdA"] = pd

                        return f

                    def mk_diagB(it):
                        def f():
                            b, h, qi = it["b"], it["h"], it["qi"]
                            q0 = b * T_ + qi * TOK_TILE
                            kb = 4 * qi
                            st = ps2.tile([128, W2], f32, tag="st", bufs=2, name="st")
                            pd = ppool.tile([128, W2], bf16, tag="pt", bufs=16, name="pd")
                            nc.tensor.matmul(
                                st[:, ds(0, 256)],
                                lhsT=kT[:, ds(b * T_ + (kb + 2) * 128, 128)],
                                rhs=qT[:, h, ds(q0 + 256, 256)],
                                start=True,
                                stop=True,
                            )
                            nc.tensor.matmul(
                                st[:, ds(256, 128)],
                                lhsT=kT[:, ds(b * T_ + (kb + 3) * 128, 128)],
                                rhs=qT[:, h, ds(q0 + 384, 128)],
                                start=True,
                                stop=True,
                            )
                            nc.scalar.activation(
                                out=pd[:, ds(0, 384)], in_=st[:, ds(0, 384)], func=AF.Exp
                            )
                            it["pdB"] = pd

                        return f

                    def mk_av_pair(it, jp):
                        def f():
                            b, qi = it["b"], it["qi"]
                            pt = it["p"][jp]
                            yps = get_yps(it)
                            for jj in range(2):
                                j = 2 * jp + jj
                                nc.tensor.matmul(
                                    yps[:],
                                    lhsT=vtm[:, b * KTB + j, :],
                                    rhs=pt[:, ds(jj * TOK_TILE, TOK_TILE)],
                                    start=(j == 0),
                                    stop=False,
                                )
                            lacc = get_lacc(it)
                            if jp == 0:
                                nc.vector.tensor_add(
                                    out=lacc[:],
                                    in0=pt[:, ds(0, 512)],
                                    in1=pt[:, ds(512, 512)],
                                )
                            else:
                                sc = scrp.tile([128, TOK_TILE], bf16, name="sc")
                                nc.vector.tensor_add(
                                    out=sc[:],
                                    in0=pt[:, ds(0, 512)],
                                    in1=pt[:, ds(512, 512)],
                                )
                                nc.vector.tensor_add(
                                    out=lacc[:], in0=lacc[:], in1=sc[:]
                                )

                        return f

                    def mk_av_diag(it):
                        def f():
                            b, qi = it["b"], it["qi"]
                            kb = 4 * qi
                            pA, pB = it["pdA"], it["pdB"]
                            # zero the causally-dead triangle (kp > qo)
                            nc.vector.tensor_mul(
                                out=pA[:, ds(0, 128)], in0=pA[:, ds(0, 128)], in1=tri01[:]
                            )
                            nc.vector.tensor_mul(
                                out=pA[:, ds(512, 128)],
                                in0=pA[:, ds(512, 128)],
                                in1=tri01[:],
                            )
                            nc.vector.tensor_mul(
                                out=pB[:, ds(0, 128)], in0=pB[:, ds(0, 128)], in1=tri01[:]
                            )
                            nc.vector.tensor_mul(
                                out=pB[:, ds(256, 128)],
                                in0=pB[:, ds(256, 128)],
                                in1=tri01[:],
                            )
                            yps = get_yps(it)
                            nc.tensor.matmul(
                                yps[:],
                                lhsT=vtm[:, b * KTB + kb, :],
                                rhs=pA[:, ds(0, 512)],
                                start=(qi == 0),
                                stop=False,
                            )
                            nc.tensor.matmul(
                                yps[:, ds(128, 384)],
                                lhsT=vtm[:, b * KTB + kb + 1, :],
                                rhs=pA[:, ds(512, 384)],
                                start=False,
                                stop=False,
                            )
                            nc.tensor.matmul(
                                yps[:, ds(256, 256)],
                                lhsT=vtm[:, b * KTB + kb + 2, :],
                                rhs=pB[:, ds(0, 256)],
                                start=False,
                                stop=False,
                            )
                            nc.tensor.matmul(
                                yps[:, ds(384, 128)],
                                lhsT=vtm[:, b * KTB + kb + 3, :],
                                rhs=pB[:, ds(256, 128)],
                                start=False,
                                stop=True,
                            )
                            lacc = get_lacc(it)
                            if it["qi"] == 0:
                                nc.vector.tensor_copy(
                                    out=lacc[:], in_=pA[:, ds(0, 512)]
                                )
                            else:
                                nc.vector.tensor_add(
                                    out=lacc[:], in0=lacc[:], in1=pA[:, ds(0, 512)]
                                )
                            nc.vector.tensor_add(
                                out=lacc[:, ds(128, 384)],
                                in0=lacc[:, ds(128, 384)],
                                in1=pA[:, ds(512, 384)],
                            )
                            nc.vector.tensor_add(
                                out=lacc[:, ds(256, 256)],
                                in0=lacc[:, ds(256, 256)],
                                in1=pB[:, ds(0, 256)],
                            )
                            nc.vector.tensor_add(
                                out=lacc[:, ds(384, 128)],
                                in0=lacc[:, ds(384, 128)],
                                in1=pB[:, ds(256, 128)],
                            )

                        return f

                    def mk_finish(it):
                        def f():
                            b, h, qi = it["b"], it["h"], it["qi"]
                            lb = scrp.tile(
                                [128, TOK_TILE], bf16, tag="lb16", name="lb"
                            )
                            nc.scalar.copy(out=lb[:], in_=it["lacc"][:])
                            lps = ps2.tile([128, TOK_TILE], f32, tag="ops", bufs=2, name="lps")
                            nc.tensor.matmul(
                                lps[:],
                                lhsT=onesb[:],
                                rhs=lb[:],
                                start=True,
                                stop=True,
                            )
                            inv = invp.tile([128, TOK_TILE], f32, tag="inv", name="inv")
                            # ~11-bit reciprocal is ample: l already carries
                            # bf16 quantization noise an order larger.
                            nc.vector.reciprocal_approx_fast(out=inv[:], in_=lps[:])
                            nc.vector.tensor_mul(
                                out=yT[:, h, ds(b * T_ + qi * TOK_TILE, TOK_TILE)],
                                in0=it["yps"][:],
                                in1=inv[:],
                            )

                        return f

                    def mk_outproj(tt, Do):
                        def f():
                            ps = ps2.tile([128, TOK_TILE], f32, tag="ops", bufs=2, name="ops")
                            for c in range(GROUP):
                                nc.tensor.matmul(
                                    ps[:],
                                    lhsT=wo_s[:, c * DC + Do, :],
                                    rhs=yT[:, c, ts(tt, TOK_TILE)],
                                    start=(c == 0),
                                    stop=(c == GROUP - 1),
                                )
                            so = stg.tile([128, TOK_TILE], bf16, name="so")
                            if Do % 2 == 0:
                                nc.vector.tensor_copy(out=so[:], in_=ps[:])
                            else:
                                nc.scalar.copy(out=so[:], in_=ps[:])
                            nc.sync.dma_start(
                                out=out_d[:, Do, ts(tt, TOK_TILE)], in_=so[:]
                            )

                        return f

                    for b in range(B):
                        # b1 descends so the kernel ends on the lightest
                        # group (shortest finish chains before the drain)
                        qorder = (
                            range(NQI) if b == 0 else range(NQI - 1, -1, -1)
                        )
                        for qi in qorder:
                            for h in range(GROUP):
                                it = {
                                    "b": b,
                                    "h": h,
                                    "qi": qi,
                                    "p": [],
                                    "pdA": None,
                                    "pdB": None,
                                    "yps": None,
                                    "lacc": None,
                                }
                                stages = [mk_pair(it, jp) for jp in range(2 * qi)]
                                stages.append(mk_diagA(it))
                                stages.append(mk_diagB(it))
                                for s in stages:
                                    s()
                                    emit_filler(2)
                                for jp in range(2 * qi):
                                    fq_attn.append(mk_av_pair(it, jp))
                                fq_attn.append(mk_av_diag(it))
                                fq_attn.append(mk_finish(it))
                            # this q-tile's out-projection becomes available
                            # once its 4 finish units are emitted; queue it.
                            tt = b * NQI + qi
                            need = len(fq_attn)
                            for Do in range(DC):
                                fq_proj.append((need, mk_outproj(tt, Do)))
                    drain_filler()

    if not nc.is_finalized():
        nc.finalize()
    return nc


def _prep_inputs(hidden_states, Wq, bq, Wk, bk, Wv, bv, Wo, bo, T_=T):
    NT_ = B * T_
    scale = 1.0 / math.sqrt(d)

    x_flat = np.asarray(hidden_states, dtype=np.float32).reshape(NT_, D)
    # xt[p, Dc, t] = x[t, Dc*128+p]
    xt = np.ascontiguousarray(
        x_flat.reshape(NT_, DC, 128).transpose(2, 1, 0)
    ).astype(BF16)

    in_maps = []
    for g in range(NC_):
        Wq_g = np.asarray(Wq[g * 512 : (g + 1) * 512, :], dtype=np.float32) * scale
        bq_g = np.asarray(bq[g * 512 : (g + 1) * 512], dtype=np.float32) * scale
        Wk_g = np.asarray(Wk[g * 128 : (g + 1) * 128, :], dtype=np.float32)
        bk_g = np.asarray(bk[g * 128 : (g + 1) * 128], dtype=np.float32)
        Wv_g = np.asarray(Wv[g * 128 : (g + 1) * 128, :], dtype=np.float32)
        bv_g = np.asarray(bv[g * 128 : (g + 1) * 128], dtype=np.float32)
        Wo_g = np.asarray(Wo[:, g * 512 : (g + 1) * 512], dtype=np.float32)

        # wq[p, dq*DC+Dc, m] = Wq_g[dq*128+m, Dc*128+p]
        wq_host = np.ascontiguousarray(
            Wq_g.reshape(GROUP, 128, DC, 128).transpose(3, 0, 2, 1).reshape(
                128, GROUP * DC, 128
            )
        ).astype(BF16)
        # wk[p, Dc, m] = Wk_g[m, Dc*128+p]
        wk_host = np.ascontiguousarray(
            Wk_g.reshape(128, DC, 128).transpose(2, 1, 0)
        ).astype(BF16)
        wv_host = np.ascontiguousarray(
            Wv_g.reshape(128, DC, 128).transpose(2, 1, 0)
        ).astype(BF16)
        # wo[p, c*DC+Do, m] = Wo_g[Do*128+m, c*128+p]
        wo_host = np.ascontiguousarray(
            Wo_g.reshape(DC, 128, GROUP, 128).transpose(3, 2, 0, 1).reshape(
                128, GROUP * DC, 128
            )
        ).astype(BF16)

        in_maps.append(
            {
                "xt": xt,
                "wq": wq_host,
                "wk": wk_host,
                "wv": wv_host,
                "wo": wo_host,
                "bq": np.ascontiguousarray(bq_g.reshape(GROUP, 128).T),
                "bk": bk_g.reshape(128, 1).copy(),
                "bv": bv_g.reshape(128, 1).copy(),
            }
        )
    return in_maps


def kernel(
    hidden_states, Wq, bq, Wk, bk, Wv, bv, Wo, bo, _trace=False, _result_box=None
):
    from concourse.bass_utils import run_bass_kernel_spmd

    if "nc" not in _program_cache:
        _program_cache["nc"] = _build_program()
    nc = _program_cache["nc"]

    in_maps = _prep_inputs(hidden_states, Wq, bq, Wk, bk, Wv, bv, Wo, bo)
    res = run_bass_kernel_spmd(
        nc, in_maps, core_ids=list(range(NC_)), trace=_trace
    )
    if _result_box is not None:
        _result_box.append(res)

    NT_ = B * T
    acc = np.zeros((128, DC, NT_), dtype=np.float32)
    for r in res.results:
        acc += r["out"].astype(np.float32)
    # outT[Do*128+p, t] = acc[p, Do, t];  out[t, :] = outT[:, t] + bo
    outT = acc.transpose(1, 0, 2).reshape(D, NT_)
    out = outT.T + np.asarray(bo, dtype=np.float32)[None, :]
    return np.ascontiguousarray(out.reshape(B, T, D), dtype=np.float32)
